# revision 13
# baseline (speedup 1.0000x reference)
"""Multi-head attention (B=2, S=2048, D=1024, H=16) on 8 Trainium2 NeuronCores.

Sharding: head-parallel attention (2 heads/core, both batches); 4 chunked
AllToAlls (one per 1024-token attention block, interleaved 128-token slots)
redistribute attention outputs so each core runs the output projection for
4x128 tokens per round, overlapped with later attention blocks.

Per-core pipeline (core c, heads hA=2c, hB=2c+1), all matmuls bf16:
  - QKV(b0) dense, then attention blocks (b, ich in 0..1, 1024 q-tokens);
    QKV(b1) + V transposes are interleaved into blocks 0-1, Wo rounds 0-2
    into blocks 2-3, to keep the PE dense (HAM stays at K=8/8).
  - scores^T per j (128 keys): two heads row-packed (tile_position (0,0)
    and (64,0)) run concurrently; one [128, 2048]-span exp on ScalarE per j
    with fused 1/8 scale writes both heads' pT.
  - AV: col-packed M=64 (head A -> psum partitions 0-63 at (0,0), head B
    -> 64-127 at (0,64)) into one shared [128,1024] accumulator; softmax
    denominators via a 4-way col-packed M=1 ones-matmul wave into one
    [128,512] bank (partitions 0/32/64/96).
  - normalize: DVE reciprocal rows + DMA partition-broadcast + DVE mul
    -> bf16; 8 slots -> DRAM -> AllToAll round -> Wo matmuls + bias.
PSUM in attention: scores 4 banks + av 2 + den 1 + overlay 1 = 8.
"""

import numpy as np

B, S, D, H, HD = 2, 2048, 1024, 16, 64
NCORES = 8
BT = B * S
SCALE = 1.0 / 8.0
BLOCKS = [(0, 0), (0, 1), (1, 0), (1, 1)]

_CACHE = {}


def _build():
    import concourse.bacc as bacc
    import concourse.tile as tile
    import concourse.mybir as mybir

    F32 = mybir.dt.float32
    BF16 = mybir.dt.bfloat16
    EXP = mybir.ActivationFunctionType.Exp

    nc = bacc.Bacc("TRN2", target_bir_lowering=False, debug=False,
                   num_devices=NCORES)

    # ---- I/O ------------------------------------------------------------
    xt = nc.dram_tensor("xt", [4, 8, 128, 1024], BF16,
                        kind="ExternalInput")
    wqkv = nc.dram_tensor("wqkv", [8, 128, 384], BF16, kind="ExternalInput")
    bq = nc.dram_tensor("bq", [128, 1], F32, kind="ExternalInput")
    bk = nc.dram_tensor("bk", [128, 1], F32, kind="ExternalInput")
    bv = nc.dram_tensor("bv", [128, 1], F32, kind="ExternalInput")
    wo = nc.dram_tensor("wo", [8, 128, D], BF16, kind="ExternalInput")
    bo = nc.dram_tensor("bo", [1, D], F32, kind="ExternalInput")
    eye = nc.dram_tensor("eye", [128, 128], BF16, kind="ExternalInput")
    ones64 = nc.dram_tensor("ones64", [128, 64], BF16, kind="ExternalInput")
    out = nc.dram_tensor("out", [512, D], F32, kind="ExternalOutput")

    KB = D // 128     # 8 contraction blocks
    groups = [list(range(NCORES))]

    with tile.TileContext(nc) as tc:
        from contextlib import ExitStack
        with ExitStack() as ctx:
            persist = ctx.enter_context(tc.tile_pool(name="persist", bufs=1))
            dram = ctx.enter_context(
                tc.tile_pool(name="dram", bufs=1, space="DRAM"))

            # ---- small constant loads (front of DMA queue) --------------
            wqkv_sb = []
            for k in range(KB):
                t = persist.tile([128, 384], BF16, tag=f"wqkv{k}",
                                 name=f"wqkv{k}")
                nc.sync.dma_start(t[:], wqkv[k])
                wqkv_sb.append(t)
            bq_sb = persist.tile([128, 1], F32, tag="bq")
            bk_sb = persist.tile([128, 1], F32, tag="bk")
            bv_sb = persist.tile([128, 1], F32, tag="bv")
            nc.sync.dma_start(bq_sb[:], bq[:])
            nc.sync.dma_start(bk_sb[:], bk[:])
            nc.sync.dma_start(bv_sb[:], bv[:])
            eye_sb = persist.tile([128, 128], BF16, tag="eye")
            nc.sync.dma_start(eye_sb[:], eye[:])
            ones_sb = persist.tile([128, 64], BF16, tag="ones64")
            nc.sync.dma_start(ones_sb[:], ones64[:])

            # persistent activations
            qT = [persist.tile([128, S], BF16, tag=f"qT{b}", name=f"qT{b}")
                  for b in range(B)]
            kT = [persist.tile([128, S], BF16, tag=f"kT{b}", name=f"kT{b}")
                  for b in range(B)]
            # V in [128 keys, 2 heads, 64 dims] layout per 128-token block
            vv = [persist.tile([128, 2, HD], BF16, tag=f"v{tb}",
                               name=f"v{tb}")
                  for tb in range(BT // 128)]

            xq_pool = ctx.enter_context(tc.tile_pool(name="xq", bufs=17))
            vt_pool = ctx.enter_context(tc.tile_pool(name="vtmp", bufs=6))

            def load_quarter(q):
                xk = []
                for k in range(KB):
                    t = xq_pool.tile([128, 1024], BF16, tag="xq",
                                     name=f"xq{q}_{k}")
                    nc.sync.dma_start(t[:], xt[q, k])
                    xk.append(t)
                return xk

            def emit_proj_group(pool, xk, ch, which):
                """One accumulation group: 8 MMs. which: 0=Q, 1=K, 2=V."""
                cs, ce = ch * 512, (ch + 1) * 512
                acc = pool.tile([128, 512], F32, tag="ov", name="ovacc")
                wcol = which * 128
                for k in range(KB):
                    nc.tensor.matmul(
                        acc[:], wqkv_sb[k][:, wcol:wcol + 128],
                        xk[k][:, cs:ce],
                        start=(k == 0), stop=(k == KB - 1))
                return acc

            def emit_proj_drain(acc, q, ch, which):
                """Bias-add PSUM->SBUF. Returns vt tile for V, else None."""
                b = q // 2
                lo = (q % 2) * 1024 + ch * 512
                if which == 0:
                    nc.vector.tensor_scalar_add(
                        qT[b][:, lo:lo + 512], acc[:], bq_sb[:])
                    return None
                if which == 1:
                    nc.vector.tensor_scalar_add(
                        kT[b][:, lo:lo + 512], acc[:], bk_sb[:])
                    return None
                vt = vt_pool.tile([128, 512], BF16, tag="vt")
                nc.vector.tensor_scalar_add(vt[:], acc[:], bv_sb[:])
                return vt

            def emit_v_transpose(pool, vt, q, ch, blk):
                """PE-transpose one 128-token block of vt into vv."""
                tb = q * 8 + ch * 4 + blk
                ovt = pool.tile([128, 512], F32, tag="ov", name="ovacc")
                pv = ovt[:].bitcast(BF16)
                nc.tensor.transpose(
                    pv[:, 0:128], vt[:, blk * 128:(blk + 1) * 128],
                    eye_sb[:])
                nc.vector.tensor_copy(
                    vv[tb][:], pv[:, 0:128].rearrange(
                        "p (h d) -> p h d", h=2))

            # ---- QKV for quarter 0 only (serial head; quarters 1-3
            # become overlay work inside the attention blocks) ------------
            xk0 = load_quarter(0)
            xk1 = load_quarter(1)
            with tc.tile_pool(name="p0_ps", bufs=3, space="PSUM") as p0:
                for ch in range(2):
                    for which in (1, 2, 0):
                        acc = emit_proj_group(p0, xk0, ch, which)
                        vt = emit_proj_drain(acc, 0, ch, which)
                        if vt is not None:
                            for blk in range(4):
                                emit_v_transpose(p0, vt, 0, ch, blk)

            xk2 = load_quarter(2)
            xk3 = load_quarter(3)
            wo_sb = []
            for r in range(NCORES):
                t = persist.tile([128, D], BF16, tag=f"wo{r}", name=f"wo{r}")
                nc.sync.dma_start(t[:], wo[r])
                wo_sb.append(t)
            bo_bc = persist.tile([128, D], F32, tag="bo_bc")
            nc.sync.dma_start(bo_bc[:], bo[:].to_broadcast((128, D)))

            # ---- attention-phase pools ----------------------------------
            ov_pool = ctx.enter_context(
                tc.tile_pool(name="ov_ps", bufs=1, space="PSUM"))
            ps_pool = ctx.enter_context(
                tc.tile_pool(name="sc_ps", bufs=1, space="PSUM"))
            av_pool = ctx.enter_context(
                tc.tile_pool(name="av_ps", bufs=1, space="PSUM"))
            den_pool = ctx.enter_context(
                tc.tile_pool(name="den_ps", bufs=1, space="PSUM"))
            pt_pool = ctx.enter_context(tc.tile_pool(name="pt", bufs=3))
            an_pool = ctx.enter_context(tc.tile_pool(name="an", bufs=2))
            rc_pool = ctx.enter_context(tc.tile_pool(name="rc", bufs=2))
            ko_pool = ctx.enter_context(tc.tile_pool(name="ko", bufs=2))
            ot_pool = ctx.enter_context(tc.tile_pool(name="osb", bufs=2))

            # ---- overlay worklists for the attention blocks -------------
            # Atoms of <=4 matmuls; paired atoms share one ov psum tile and
            # stay adjacent in the work list (only attention MMs, which use
            # other banks, may run between them).
            vt_hold = []  # V-proj drains of b1, consumed by transposes

            def ov_qkv(xk, q, ch, which):
                cs, ce = ch * 512, (ch + 1) * 512
                wcol = which * 128
                hold = {}

                def a1():
                    acc = ov_pool.tile([128, 512], F32, tag="ov",
                                       name="ovacc")
                    for k in range(4):
                        nc.tensor.matmul(
                            acc[:], wqkv_sb[k][:, wcol:wcol + 128],
                            xk[k][:, cs:ce], start=(k == 0), stop=False)
                    hold[0] = acc

                def a2():
                    acc = hold[0]
                    for k in range(4, KB):
                        nc.tensor.matmul(
                            acc[:], wqkv_sb[k][:, wcol:wcol + 128],
                            xk[k][:, cs:ce], start=False, stop=(k == KB - 1))
                    vt = emit_proj_drain(acc, q, ch, which)
                    if vt is not None:
                        vt_hold.append((vt, q, ch))
                return [a1, a2]

            def ov_vtrans(idx, blk):
                def emit():
                    vt, q, ch = vt_hold[idx]
                    emit_v_transpose(ov_pool, vt, q, ch, blk)
                return emit

            ov_work = {0: [], 1: [], 2: [], 3: []}
            vt_idx = [0]

            def add_quarter(dst, q, xk, with_q=True):
                for ch in range(2):
                    for which in (1, 2) if not with_q else (1, 2, 0):
                        ov_work[dst].extend(ov_qkv(xk, q, ch, which))
                        if which == 2:
                            idx = vt_idx[0]
                            vt_idx[0] += 1
                            for blk in range(4):
                                ov_work[dst].append(ov_vtrans(idx, blk))

            # block 0: rest of batch 0 (quarter 1); block 1: quarter 2 +
            # Q of quarter 3; block 2 (first half): K/V of quarter 3.
            add_quarter(0, 1, xk1)
            add_quarter(1, 2, xk2)
            for ch in range(2):
                ov_work[1].extend(ov_qkv(xk3, 3, ch, 0))
            add_quarter(2, 3, xk3, with_q=False)

            # ---- exchange + Wo machinery --------------------------------
            a2a_in = [dram.tile([NCORES, 128, 128], BF16,
                                name=f"a2a_in{r}") for r in range(4)]
            a2a_out = [dram.tile([NCORES, 128, 128], BF16,
                                 name=f"a2a_out{r}") for r in range(4)]

            def ov_wo_round(rnd):
                """Wo chunk closures for one received round."""
                chunks = []
                ko_hold = {}

                def emit_load():
                    t = ko_pool.tile([128, NCORES, 128], BF16, tag="ko")
                    for s in range(NCORES):
                        nc.sync.dma_start(t[:, s, :], a2a_out[rnd][s])
                    ko_hold[0] = t
                chunks.append(emit_load)

                def half(nh):
                    hold = {}

                    def a1():
                        ko = ko_hold[0]
                        acc = ov_pool.tile([128, 512], F32, tag="ov",
                                           name="ovacc")
                        for s in range(4):
                            nc.tensor.matmul(
                                acc[:], ko[:, s, :],
                                wo_sb[s][:, nh * 512:(nh + 1) * 512],
                                start=(s == 0), stop=False)
                        hold[0] = acc

                    def a2():
                        ko = ko_hold[0]
                        acc = hold[0]
                        for s in range(4, NCORES):
                            nc.tensor.matmul(
                                acc[:], ko[:, s, :],
                                wo_sb[s][:, nh * 512:(nh + 1) * 512],
                                start=False, stop=(s == NCORES - 1))
                        ot = ot_pool.tile([128, 512], F32, tag="ot")
                        nc.vector.tensor_add(
                            ot[:], acc[:], bo_bc[:, nh * 512:(nh + 1) * 512])
                        nc.sync.dma_start(
                            out[rnd * 128:(rnd + 1) * 128,
                                nh * 512:(nh + 1) * 512], ot[:])
                    return [a1, a2]
                chunks.extend(half(0))
                chunks.extend(half(1))
                return chunks

            # Wo rounds 0,1 overlay block 2; round 2 overlays block 3.
            ov_work[2].extend(ov_wo_round(0))
            ov_work[2].extend(ov_wo_round(1))
            ov_work[3].extend(ov_wo_round(2))

            # ---- attention blocks ---------------------------------------
            def emit_scores(b, ich, j):
                """4 score MMs (row-packed pairs) + the 2 per-head exps."""
                qlo = ich * 1024
                klo = j * 128
                pts = []
                for h in range(2):
                    psh = ps_pool.tile([128, 1024], F32, tag=f"ps{h}",
                                       name=f"ps{h}")
                    for sub in range(2):
                        nc.tensor.matmul(
                            psh[:, sub * 512:(sub + 1) * 512],
                            kT[b][h * 64:(h + 1) * 64, klo:klo + 128],
                            qT[b][h * 64:(h + 1) * 64,
                                  qlo + sub * 512:qlo + (sub + 1) * 512],
                            start=True, stop=True,
                            tile_position=(h * 64, 0))
                    pth = pt_pool.tile([128, 1024], BF16, tag=f"pt{h}",
                                       name=f"pt{h}")
                    nc.scalar.activation(pth[:], psh[:], EXP, scale=SCALE)
                    pts.append(pth)
                return pts

            carry = None
            for blk, (b, ich) in enumerate(BLOCKS):
                av = av_pool.tile([128, 1024], F32, tag="av")
                den = den_pool.tile([128, 512], F32, tag="den")
                work = list(ov_work[blk])
                nv = len(work)

                def emit_av_den(j, pts, av=av, den=den, b=b):
                    tb = b * 16 + j
                    for sub in range(2):
                        lo, hi = sub * 512, (sub + 1) * 512
                        nc.tensor.matmul(
                            av[0:64, lo:hi], vv[tb][:, 0, :],
                            pts[0][:, lo:hi], start=(j == 0), stop=(j == 15),
                            tile_position=(0, 0))
                        nc.tensor.matmul(
                            av[64:128, lo:hi], vv[tb][:, 1, :],
                            pts[1][:, lo:hi], start=(j == 0), stop=(j == 15),
                            tile_position=(0, 64))
                    for h in range(2):
                        for sub in range(2):
                            tp = h * 64 + sub * 32
                            nc.tensor.matmul(
                                den[tp:tp + 1, :], ones_sb[:, 0:1],
                                pts[h][:, sub * 512:(sub + 1) * 512],
                                start=(j == 0), stop=(j == 15),
                                tile_position=(0, tp))

                hist = {}
                j0 = 0
                if carry is not None:
                    hist[0], hist[1] = carry
                    j0 = 2
                nslots = 16 - j0
                for j in range(j0, 16):
                    hist[j] = emit_scores(b, ich, j)
                    # AV/den lag two j's behind so next j's scores sit
                    # directly behind this j's in the PE queue
                    if j >= 2:
                        emit_av_den(j - 2, hist.pop(j - 2))
                    # overlay chunk(s) for this j-slot
                    sl = j - j0
                    for wi in range((sl * nv) // nslots,
                                    ((sl + 1) * nv) // nslots):
                        work[wi]()
                emit_av_den(14, hist.pop(14))
                emit_av_den(15, hist.pop(15))
                # hoist the next block's first two score/exp waves so the
                # ACT queue stays saturated across the block boundary
                if blk + 1 < len(BLOCKS):
                    nb, nich = BLOCKS[blk + 1]
                    carry = (emit_scores(nb, nich, 0),
                             emit_scores(nb, nich, 1))

                # normalize: broadcast den rows across partitions via K=1
                # ones-matmuls, reciprocal at base 0, multiply to bf16.
                # (reciprocal_approx_fast and partition_broadcast are both
                # broken for APs at base partition != 0.)
                den_sb = rc_pool.tile([128, 512], BF16, tag="den_sb")
                nc.vector.tensor_copy(den_sb[:], den[:])
                an = an_pool.tile([128, 1024], BF16, tag="an")
                for sub in range(2):
                    ovn = ov_pool.tile([128, 512], F32, tag="ov",
                                       name="ovacc")
                    for h in range(2):
                        tp = h * 64 + sub * 32
                        nc.tensor.matmul(
                            ovn[h * 64:(h + 1) * 64, :],
                            ones_sb[tp:tp + 1, :],
                            den_sb[tp:tp + 1, :], start=True, stop=True,
                            tile_position=(tp, h * 64))
                    rec = rc_pool.tile([128, 512], F32, tag="rec")
                    nc.vector.reciprocal_approx_fast(rec[:], ovn[:])
                    nc.vector.tensor_mul(
                        an[:, sub * 512:(sub + 1) * 512],
                        av[:, sub * 512:(sub + 1) * 512], rec[:])
                for p in range(NCORES):
                    nc.sync.dma_start(a2a_in[blk][p],
                                      an[:, p * 128:(p + 1) * 128])
                nc.gpsimd.collective_compute(
                    "AllToAll", mybir.AluOpType.bypass,
                    ins=[a2a_in[blk][:]], outs=[a2a_out[blk][:]],
                    replica_groups=groups)

            # ---- tail: keep the PE warm while round 3 is in flight,
            # then Wo round 3 ---------------------------------------------
            tail_chunks = ov_wo_round(3)
            tail_chunks[0]()  # ko DMA loads (wait on the collective)
            for i in range(14):
                dmy = ov_pool.tile([128, 512], F32, tag="ov", name="ovacc")
                nc.tensor.matmul(dmy[:], wqkv_sb[0][:, 0:128],
                                 qT[0][:, 0:512], start=True, stop=True)
            for emit in tail_chunks[1:]:
                emit()

    nc.compile()
    return nc


def _get_nc():
    if "nc" not in _CACHE:
        _CACHE["nc"] = _build()
    return _CACHE["nc"]


def _make_in_maps(hidden_states, Wq, bq, Wk, bk, Wv, bv, Wo, bo):
    import ml_dtypes
    bf16 = ml_dtypes.bfloat16
    hs = np.ascontiguousarray(np.asarray(hidden_states, dtype=np.float32))
    xT = hs.reshape(BT, D).T.astype(bf16)
    # pre-tiled: [quarter, kblock, 128, 1024] contiguous
    xt = np.ascontiguousarray(
        xT.reshape(8, 128, 4, 1024).transpose(2, 0, 1, 3))
    eye = np.eye(128, dtype=bf16)
    ones64 = np.ones((128, 64), dtype=bf16)
    Wq = np.asarray(Wq, np.float32).astype(bf16)
    Wk = np.asarray(Wk, np.float32).astype(bf16)
    Wv = np.asarray(Wv, np.float32).astype(bf16)
    Wo = np.asarray(Wo, np.float32).astype(bf16)
    Wo_t = np.ascontiguousarray(Wo.reshape(8, 128, D))
    bq = np.asarray(bq, np.float32); bk = np.asarray(bk, np.float32)
    bv = np.asarray(bv, np.float32); bo = np.asarray(bo, np.float32)
    in_maps = []
    for c in range(NCORES):
        sl = slice(2 * c * HD, (2 * c + 2) * HD)
        wqkv_c = np.concatenate([Wq[:, sl], Wk[:, sl], Wv[:, sl]],
                                axis=1)  # [D, 384]
        wqkv_t = np.ascontiguousarray(wqkv_c.reshape(8, 128, 384))
        in_maps.append({
            "xt": xt,
            "wqkv": wqkv_t,
            "bq": np.ascontiguousarray(bq[sl].reshape(128, 1)),
            "bk": np.ascontiguousarray(bk[sl].reshape(128, 1)),
            "bv": np.ascontiguousarray(bv[sl].reshape(128, 1)),
            "wo": Wo_t,
            "bo": np.ascontiguousarray(bo.reshape(1, D)),
            "eye": eye,
            "ones64": ones64,
        })
    return in_maps


def run(trace=False, tmpdir=None, **inputs):
    from concourse.bass_utils import run_bass_kernel_spmd
    nc = _get_nc()
    in_maps = _make_in_maps(**inputs)
    res = run_bass_kernel_spmd(nc, in_maps, list(range(NCORES)), trace=trace,
                               tmpdir=tmpdir)
    full = np.empty((B, S, D), dtype=np.float32)
    for c in range(NCORES):
        o = res.results[c]["out"]
        for rnd, (b, ich) in enumerate(BLOCKS):
            t0 = ich * 1024 + c * 128
            full[b, t0:t0 + 128, :] = o[rnd * 128:(rnd + 1) * 128]
    return full, res


def kernel(**inputs) -> np.ndarray:
    out, _ = run(trace=False, **inputs)
    return out
